# revision 15
# baseline (speedup 1.0000x reference)
"""Trainium2 Bass kernel for EvolveGCN-O forward (GCN message passing).

Math (reference):
    h   = x @ Wp + bp
    W   = LSTM-evolved weight from initial_weight (one step, h0=c0=IW)
    hw  = h @ W
    out = D^-1/2 (A+I) D^-1/2 hw + b_gcn

Factored for the kernel:
    out[d] = dinv[d] * (sum_{e: dst=d} dinv[src_e] * x[src_e]) @ (Wp @ W)
             + s2[d]*dinv[d]*(bp @ W) + b_gcn
with s2[d] = sum_{e in(d)} dinv[src_e] (self loops included as edges).

Distribution: nodes (dsts) sharded over 8 NeuronCores (serpentine by degree).
The host pre-scales x by dinv[src] and uploads a fp16 token table split in 4
rank-chunks (separate SBUF tiles so tile-granular deps release early); each
core gathers its edges' source rows via SWDGE dma_gather (4 queues, calls
merged across dst-groups for chunks 0-2, per-group for the last chunk so the
tail pipelines), and accumulates per-dst-group segment sums in PSUM via
one-hot mask matmuls on the TensorEngine, adding chunk partials into
per-group SBUF accumulators. The tiny evolved-weight matmul is replicated.
"""

import numpy as np

N_NODES = 10000
N_EDGES = 320000
IN_DIM = 128
HID = 256
M = 8                    # NeuronCores
NP = 10240               # padded node count (mult of 128)
RANKS = NP // 128        # 80
NPC = NP // M            # 1280 padded dsts per core
NGRP = NPC // 128        # 10 dst blocks of 128 per core
NCHK = 4                 # src-token chunks of the table
RC = RANKS // NCHK       # 20 ranks per chunk
CHTOK = RC * 128         # 2560 tokens per chunk
NCALL = 4                # equal gather calls per chunk (k < NCHK-1)
NCALL3 = 16              # smaller calls in the last chunk (tail drain)
DUMMY_DL = 999.0         # dst_local value that matches no column

_cache = {}


def _round_up(x, m):
    return (x + m - 1) // m * m


def _plan_calls(nidx):
    """Partition each chunk's slot stream into equal-size gather calls.

    Returns plan[k] = list of (pieces, call_nidx); piece = (g, off_in_call,
    ni_piece, is_last_piece_of_segment). Group segments are laid out in
    g-order and cut at 16-idx granularity; pieces may cross groups.
    Chunks 0..NCHK-2 use NCALL equal calls; the last chunk uses smaller
    calls (~NCALL3 of them) so the tail drains quickly.
    """
    plan = []
    for k in range(NCHK):
        segs = [(g, int(nidx[g][k])) for g in range(NGRP) if nidx[g][k] > 0]
        tot = sum(n for _, n in segs)
        ncall = NCALL if k < NCHK - 1 else NCALL3
        if tot == 0:
            plan.append([])
            continue
        cuts = [_round_up(tot * i // ncall, 16) for i in range(ncall + 1)]
        cuts[-1] = tot
        calls = []
        gi = 0          # current segment index
        gdone = 0       # idx consumed of current segment
        for ci in range(ncall):
            c0, c1 = cuts[ci], cuts[ci + 1]
            if c1 <= c0:
                continue
            pieces = []
            pos = c0
            while pos < c1:
                g, gn = segs[gi]
                take = min(gn - gdone, c1 - pos)
                last = (gdone + take == gn)
                pieces.append((g, pos - c0, take, last))
                pos += take
                gdone += take
                if last and pos <= c1:
                    gi += 1
                    gdone = 0
                    if gi >= len(segs):
                        break
            calls.append((pieces, c1 - c0))
        plan.append(calls)
    return plan


def _mask_cols(nidx, plan):
    """Call-relative mask column layout: (k,g,pidx) -> (c0, ncol, srel).

    pidx counts the pieces of (k, g) in emission order.
    """
    mcols = {}
    for k in range(NCHK):
        pcount = {}
        for pieces, cni in plan[k]:
            for g, off, ni, last in pieces:
                pidx = pcount.get(g, 0)
                pcount[g] = pidx + 1
                c0 = off // 128
                c1 = -(-(off + ni) // 128)
                mcols[(k, g, pidx)] = (c0, c1 - c0, off - 128 * c0)
    return mcols


def _build_module(nidx):
    """Build+compile the Bacc module.

    nidx[g][k] — gather idx count per (dst-group, src-chunk), mult of 16.
    """
    import concourse.bacc as bacc
    import concourse.mybir as mybir
    import concourse.tile as tile

    plan = _plan_calls(nidx)
    mcols = _mask_cols(nidx, plan)
    TOTI = int(sum(sum(r) for r in nidx))
    TOTC = int(sum(v[1] for v in mcols.values()))

    nc = bacc.Bacc("TRN2", target_bir_lowering=False, debug=False,
                   num_devices=M, num_swdge_queues=4)
    f32, f16, i16 = mybir.dt.float32, mybir.dt.float16, mybir.dt.int16

    # ---- DRAM inputs ----
    xs_ins = [nc.dram_tensor(f"xs{k}", [128, CHTOK], f16, kind="ExternalInput").ap()
              for k in range(NCHK)]
    idx_in = nc.dram_tensor("idx", [128, TOTI // 16], i16, kind="ExternalInput").ap()
    dla_in = nc.dram_tensor("dla", [128, TOTC], f16, kind="ExternalInput").ap()
    iota_in = nc.dram_tensor("iota", [128, 128], f16, kind="ExternalInput").ap()
    wiht_in = nc.dram_tensor("W_ihT", [256, 1024], f32, kind="ExternalInput").ap()
    whht_in = nc.dram_tensor("W_hhT", [256, 1024], f32, kind="ExternalInput").ap()
    iw_in = nc.dram_tensor("IW", [256, 256], f32, kind="ExternalInput").ap()
    iwt_in = nc.dram_tensor("IWT", [256, 256], f32, kind="ExternalInput").ap()
    wpt_in = nc.dram_tensor("WpT", [256, 128], f32, kind="ExternalInput").ap()
    bih_in = nc.dram_tensor("b_ih", [1, 1024], f32, kind="ExternalInput").ap()
    bhh_in = nc.dram_tensor("b_hh", [1, 1024], f32, kind="ExternalInput").ap()
    bp_in = nc.dram_tensor("bp_col", [256, 1], f32, kind="ExternalInput").ap()
    bgcn_in = nc.dram_tensor("b_gcn", [1, 256], f32, kind="ExternalInput").ap()
    ones_in = nc.dram_tensor("ones_row", [1, 128], f32, kind="ExternalInput").ap()
    s2_in = nc.dram_tensor("s2_row", [1, NPC], f32, kind="ExternalInput").ap()
    dri_in = nc.dram_tensor("dri_row", [1, NPC], f32, kind="ExternalInput").ap()
    dcol_in = nc.dram_tensor("dinv_col", [128, NGRP], f32, kind="ExternalInput").ap()

    out_t = nc.dram_tensor("out", [NPC, HID], f32, kind="ExternalOutput").ap()

    def gather_sbuf_nt(out_ap, in_ap, idxs_ap, num_idxs, queue_num):
        eng = nc.gpsimd
        in_ap = in_ap.bitcast(out_ap.dtype) if in_ap.dtype != out_ap.dtype else in_ap
        return eng.add_instruction(
            mybir.InstDMAGatherAnt(
                name=f"I-{nc.next_id()}",
                ins=[eng.lower_ap(in_ap), eng.lower_ap(idxs_ap),
                     eng.lower_val_access(eng.to_reg(num_idxs))],
                outs=[eng.lower_ap(out_ap)],
                transpose=False,
                num_idxs=num_idxs,
                elem_size=128,
                stride_bytes_256=0,
                gen_mode=0,
                single_packet=False,
                queue_num=queue_num,
                sbuf_tokens_per_rank=128,
                sbuf_free_dim_per_rank=256,
                sbuf_free_dim_pad_per_rank=0,
                sbuf_byte_offset=0,
            )
        )

    from concourse import library_config

    with tile.TileContext(nc) as tc:
        nc.gpsimd.load_library(library_config.mlp)
        with (
            tc.tile_pool(name="stage", bufs=1) as stpool,
            tc.tile_pool(name="persist", bufs=1) as pp,
            tc.tile_pool(name="gp", bufs=6) as gpool,
            tc.tile_pool(name="sp", bufs=10) as spool,
            tc.tile_pool(name="fin", bufs=4) as fpool,
            tc.tile_pool(name="psacc", bufs=3, space="PSUM") as psacc,
            tc.tile_pool(name="psg", bufs=3, space="PSUM") as psg,
            tc.tile_pool(name="psl", bufs=1, space="PSUM") as psl,
        ):
            # ---------- phase 0: per-chunk loads (tile-granular deps) ----
            ick = [sum(int(nidx[g][k]) for g in range(NGRP)) // 16
                   for k in range(NCHK)]
            ioffs = [0]
            for k in range(NCHK):
                ioffs.append(ioffs[-1] + ick[k])
            cck = [sum(v[1] for key, v in mcols.items() if key[0] == k)
                   for k in range(NCHK)]
            coffs = [0]
            for k in range(NCHK):
                coffs.append(coffs[-1] + cck[k])

            xs_c = []
            idx_c = []
            dla_c = []
            for k in range(NCHK):
                t_xs = pp.tile([128, CHTOK], f16, name=f"xs_c{k}")
                xs_c.append(t_xs)
                t_ix = pp.tile([128, max(ick[k], 1)], i16, name=f"idx_c{k}")
                idx_c.append(t_ix)
                t_dl = pp.tile([128, max(cck[k], 1)], f16, name=f"dla_c{k}")
                dla_c.append(t_dl)
            iota = pp.tile([128, 128], f16)

            for k in range(NCHK):
                hm = CHTOK // 2
                if ick[k]:
                    nc.scalar.dma_start(out=idx_c[k][:],
                                        in_=idx_in[:, ioffs[k]:ioffs[k + 1]])
                nc.sync.dma_start(out=xs_c[k][:, 0:hm], in_=xs_ins[k][:, 0:hm])
                nc.scalar.dma_start(out=xs_c[k][:, hm:CHTOK],
                                    in_=xs_ins[k][:, hm:CHTOK])
                if cck[k]:
                    nc.scalar.dma_start(out=dla_c[k][:],
                                        in_=dla_in[:, coffs[k]:coffs[k + 1]])
                if k == 0:
                    nc.scalar.dma_start(out=iota[:], in_=iota_in[:])

            def emit_weights():
                # ---------- small-tensor loads ----------
                wiht = pp.tile([128, 2, 1024], f32)
                whht = pp.tile([128, 2, 1024], f32)
                iw = pp.tile([128, 2, 256], f32)
                iwt = pp.tile([128, 2, 256], f32)
                wpt = pp.tile([128, 2, 128], f32)
                bih = pp.tile([1, 1024], f32)
                bhh = pp.tile([1, 1024], f32)
                bp_c = pp.tile([128, 2, 1], f32)
                bgcn = pp.tile([1, 256], f32)
                ones = pp.tile([1, 128], f32)
                s2r = pp.tile([1, NPC], f32)
                drir = pp.tile([1, NPC], f32)
                dcol = pp.tile([128, NGRP], f32)
                for t_, s_ in ((wiht, wiht_in), (whht, whht_in), (iw, iw_in),
                               (iwt, iwt_in), (wpt, wpt_in), (bp_c, bp_in)):
                    nc.sync.dma_start(out=t_[:], in_=s_.rearrange("(k p) c -> p k c", p=128))
                for t_, s_ in ((bih, bih_in), (bhh, bhh_in), (bgcn, bgcn_in),
                               (ones, ones_in), (s2r, s2_in), (drir, dri_in),
                               (dcol, dcol_in)):
                    nc.sync.dma_start(out=t_[:], in_=s_[:])

                # ---------- LSTM weight evolution (tiny, replicated) ----------
                wsum = pp.tile([128, 2, 1024], f32)
                nc.vector.tensor_tensor(out=wsum[:], in0=wiht[:], in1=whht[:],
                                        op=mybir.AluOpType.add)
                bsum = pp.tile([1, 1024], f32)
                nc.vector.tensor_tensor(out=bsum[:], in0=bih[:], in1=bhh[:],
                                        op=mybir.AluOpType.add)
                w_ev = pp.tile([128, 2, 256], f32)   # evolved GCN weight W
                for ic in range(2):
                    gpsum = psl.tile([128, 1024], f32, space="PSUM", tag="gates")
                    for h in range(2):
                        gs = slice(512 * h, 512 * (h + 1))
                        nc.tensor.matmul(out=gpsum[:, gs], lhsT=ones[:, :],
                                         rhs=bsum[:, gs], start=True, stop=False)
                        nc.tensor.matmul(out=gpsum[:, gs],
                                         lhsT=iwt[:, 0, 128 * ic:128 * (ic + 1)],
                                         rhs=wsum[:, 0, gs], start=False, stop=False)
                        nc.tensor.matmul(out=gpsum[:, gs],
                                         lhsT=iwt[:, 1, 128 * ic:128 * (ic + 1)],
                                         rhs=wsum[:, 1, gs], start=False, stop=True)
                    Sig = mybir.ActivationFunctionType.Sigmoid
                    Tanh = mybir.ActivationFunctionType.Tanh
                    si = stpool.tile([128, 256], f32, tag="si")
                    sf = stpool.tile([128, 256], f32, tag="sf")
                    tg = stpool.tile([128, 256], f32, tag="tg")
                    so = stpool.tile([128, 256], f32, tag="so")
                    nc.scalar.activation(out=si[:], in_=gpsum[:, 0:256], func=Sig)
                    nc.scalar.activation(out=sf[:], in_=gpsum[:, 256:512], func=Sig)
                    nc.scalar.activation(out=tg[:], in_=gpsum[:, 512:768], func=Tanh)
                    nc.scalar.activation(out=so[:], in_=gpsum[:, 768:1024], func=Sig)
                    c1 = stpool.tile([128, 256], f32, tag="c1")
                    nc.vector.tensor_tensor(out=c1[:], in0=sf[:],
                                            in1=iw[:, ic, :],
                                            op=mybir.AluOpType.mult)
                    c2 = stpool.tile([128, 256], f32, tag="c2")
                    nc.vector.tensor_tensor(out=c2[:], in0=si[:], in1=tg[:],
                                            op=mybir.AluOpType.mult)
                    cc = stpool.tile([128, 256], f32, tag="cc")
                    nc.vector.tensor_tensor(out=cc[:], in0=c1[:], in1=c2[:],
                                            op=mybir.AluOpType.add)
                    tcc = stpool.tile([128, 256], f32, tag="tcc")
                    nc.scalar.activation(out=tcc[:], in_=cc[:], func=Tanh)
                    nc.vector.tensor_tensor(out=w_ev[:, ic, :],
                                            in0=so[:], in1=tcc[:],
                                            op=mybir.AluOpType.mult)

                wpw = pp.tile([128, 256], f32)       # Wp @ W
                wp_ps = psg.tile([128, HID], f32, space="PSUM", tag="ops")
                nc.tensor.matmul(out=wp_ps[:], lhsT=wpt[:, 0, :], rhs=w_ev[:, 0, :],
                                 start=True, stop=False)
                nc.tensor.matmul(out=wp_ps[:], lhsT=wpt[:, 1, :], rhs=w_ev[:, 1, :],
                                 start=False, stop=True)
                nc.vector.tensor_copy(out=wpw[:], in_=wp_ps[:])
                bpw = pp.tile([1, 256], f32)         # bp @ W
                bp_ps = psg.tile([128, HID], f32, space="PSUM", tag="ops")
                nc.tensor.matmul(out=bp_ps[0:1, :], lhsT=bp_c[:, 0, :], rhs=w_ev[:, 0, :],
                                 start=True, stop=False)
                nc.tensor.matmul(out=bp_ps[0:1, :], lhsT=bp_c[:, 1, :], rhs=w_ev[:, 1, :],
                                 start=False, stop=True)
                nc.vector.tensor_copy(out=bpw[:], in_=bp_ps[0:1, :])

                return s2r, drir, dcol, wpw, bpw, bgcn

            # per-group SBUF accumulators [feat, dst]
            xaggs = []
            for g in range(NGRP):
                t_xa = pp.tile([128, 128], f32, name=f"xagg{g}")
                xaggs.append(t_xa)

            lastk = [max((k for k in range(NCHK) if nidx[g][k] > 0), default=-1)
                     for g in range(NGRP)]

            def emit_final(g, s2r, drir, dcol, wpw, bpw, bgcn):
                ops = psg.tile([128, HID], f32, space="PSUM", tag="ops")
                ds = slice(128 * g, 128 * (g + 1))
                nc.tensor.matmul(out=ops[:], lhsT=s2r[:, ds], rhs=bpw[:],
                                 start=True, stop=False)
                nc.tensor.matmul(out=ops[:], lhsT=drir[:, ds], rhs=bgcn[:],
                                 start=False, stop=False)
                nc.tensor.matmul(out=ops[:], lhsT=xaggs[g][:], rhs=wpw[:],
                                 start=False, stop=True)
                orow = fpool.tile([128, HID], f32, tag="orow")
                nc.scalar.activation(out=orow[:], in_=ops[:],
                                     func=mybir.ActivationFunctionType.Copy,
                                     scale=dcol[:, g:g + 1])
                nc.sync.dma_start(
                    out=out_t.rearrange("(g p) h -> g p h", p=128)[g],
                    in_=orow[:],
                )

            # ---------- main: equal gathers + per-piece mask matmuls ----
            qrr = [0]
            weights = None
            ioff_k = [0] * NCHK   # per-chunk idx consumption (16-units)
            coff_k = [0] * NCHK   # per-chunk dla col consumption
            started = [False] * NGRP
            for k in range(NCHK):
                pcount = {}
                for pieces, cni in plan[k]:
                    ccols = -(-cni // 128)
                    gbuf = gpool.tile([128, ccols, 128], f16, tag="gbuf")
                    qn = qrr[0] % 4
                    qrr[0] += 1
                    gather_sbuf_nt(
                        gbuf[:, :, :],
                        xs_c[k][:],
                        idx_c[k][:, ioff_k[k]:ioff_k[k] + cni // 16],
                        cni,
                        qn,
                    )
                    ioff_k[k] += cni // 16
                    if weights is None:
                        weights = emit_weights()
                    for g, off, ni, lastp in pieces:
                        pidx = pcount.get(g, 0)
                        pcount[g] = pidx + 1
                        c0, ncol, srel = mcols[(k, g, pidx)]
                        st = spool.tile([128, ncol, 128], f16, tag="st")
                        nc.vector.tensor_tensor(
                            out=st[:],
                            in0=dla_c[k][:, coff_k[k]:coff_k[k] + ncol]
                                .rearrange("p (c o) -> p c o", o=1)
                                .to_broadcast([128, ncol, 128]),
                            in1=iota[:].rearrange("p (o d) -> p o d", o=1)
                                .to_broadcast([128, ncol, 128]),
                            op=mybir.AluOpType.is_equal,
                        )
                        coff_k[k] += ncol
                        gps = psacc.tile([128, 128], f32, space="PSUM", tag="agg")
                        for c in range(ncol):
                            rem = min(128, cni - 128 * (c0 + c))
                            nc.tensor.matmul(
                                out=gps[:],
                                lhsT=gbuf[0:rem, c0 + c, :],
                                rhs=st[0:rem, c, :],
                                start=(c == 0),
                                stop=(c == ncol - 1),
                            )
                        if not started[g]:
                            started[g] = True
                            nc.scalar.activation(
                                out=xaggs[g][:], in_=gps[:],
                                func=mybir.ActivationFunctionType.Copy)
                        else:
                            nc.vector.tensor_tensor(
                                out=xaggs[g][:], in0=xaggs[g][:], in1=gps[:],
                                op=mybir.AluOpType.add)
                        if k == lastk[g] and lastp:
                            emit_final(g, *weights)

    nc.compile()
    return nc


def _preprocess(edge_index):
    """Host-side index preprocessing. Returns per-core index structures."""
    src = np.asarray(edge_index[0], dtype=np.int64)
    dst = np.asarray(edge_index[1], dtype=np.int64)
    loops = np.arange(N_NODES, dtype=np.int64)
    src_all = np.concatenate([src, loops])
    dst_all = np.concatenate([dst, loops])
    deg = np.bincount(dst_all, minlength=N_NODES).astype(np.float64)
    dinv = (1.0 / np.sqrt(deg)).astype(np.float32)

    # serpentine assignment of degree-sorted nodes to cores
    order = np.argsort(-deg, kind="stable")
    r = np.arange(N_NODES)
    rr = r % (2 * M)
    core_r = np.where(rr < M, rr, 2 * M - 1 - rr)
    lrank_r = (r // (2 * M)) * 2 + (rr >= M)
    core_of = np.empty(N_NODES, np.int64)
    lrank_of = np.empty(N_NODES, np.int64)
    core_of[order] = core_r
    lrank_of[order] = lrank_r

    # per-core permutation: perm[c][l] = global node at local rank l
    perm = np.empty((M, N_NODES // M), np.int64)
    perm[core_of[order], lrank_of[order]] = order

    # edges keyed by (core, group, chunk, src token)
    e_core = core_of[dst_all]
    e_lrank = lrank_of[dst_all]
    e_grp = e_lrank // 128
    e_dl = (e_lrank % 128).astype(np.int64)
    e_tok = src_all
    e_chk = e_tok // CHTOK
    key = ((e_core * NGRP + e_grp) * NCHK + e_chk) * NP + e_tok
    eorder = np.argsort(key, kind="stable")
    sk = key[eorder]
    s_tok = e_tok[eorder]
    s_dl = e_dl[eorder]
    s_seg = sk // NP

    NSEG = M * NGRP * NCHK
    bounds = np.searchsorted(s_seg, np.arange(NSEG + 1), side="left")
    cnt = (bounds[1:] - bounds[:-1]).reshape(M, NGRP, NCHK)
    nidx = _round_up(cnt.max(axis=0), 16)          # [NGRP, NCHK]
    TOTI = int(nidx.sum())

    plan = _plan_calls(nidx)
    mcols = _mask_cols(nidx, plan)
    TOTC = int(sum(v[1] for v in mcols.values()))

    idx_w = np.zeros((M, 16, TOTI // 16), np.int16)
    dla_t = np.full((M, 128, TOTC), DUMMY_DL, np.float16)
    # idx stream: chunk-major, group-order, globally sequential
    goff = 0
    seg_cache = {}
    for k in range(NCHK):
        for g in range(NGRP):
            ni = int(nidx[g][k])
            if ni == 0:
                continue
            i = np.arange(ni)
            for c in range(M):
                sseg = (c * NGRP + g) * NCHK + k
                b0, b1 = bounds[sseg], bounds[sseg + 1]
                n = b1 - b0
                tok_pad = np.zeros(ni, np.int64)
                tok_pad[:n] = s_tok[b0:b1] - k * CHTOK
                da = np.full(ni, DUMMY_DL, np.float64)
                da[:n] = s_dl[b0:b1]
                idx_w[c, (goff + i) % 16, (goff + i) // 16] = tok_pad.astype(np.int16)
                seg_cache[(k, g, c)] = da
            goff += ni
    # dla: per piece in emission order
    coff = 0
    for k in range(NCHK):
        pcount = {}
        pstart = {}
        for pieces, cni in plan[k]:
            for g, off, ni, lastp in pieces:
                pidx = pcount.get(g, 0)
                pcount[g] = pidx + 1
                ps = pstart.get(g, 0)
                pstart[g] = ps + ni
                _, ncol, srel = mcols[(k, g, pidx)]
                j = srel + np.arange(ni)
                for c in range(M):
                    da = seg_cache[(k, g, c)][ps:ps + ni]
                    dla_t[c, j % 128, coff + j // 128] = da.astype(np.float16)
                coff += ncol
        idx_rep = np.tile(idx_w, (1, 8, 1))

    # s2[d] = sum over in-edges of dinv[src] (self loop included)
    s2 = np.bincount(dst_all, weights=dinv[src_all].astype(np.float64),
                     minlength=N_NODES).astype(np.float32)

    return dict(dinv=dinv, perm=perm, nidx=nidx, idx_rep=idx_rep,
                dla_t=dla_t, s2=s2)


LAST_RESULT = None


def kernel(x, edge_index, Wp, bp, W_ih, W_hh, b_ih, b_hh, initial_weight, b_gcn):
    global LAST_RESULT
    from concourse.bass_utils import run_bass_kernel_spmd

    x = np.asarray(x, np.float32)
    Wp = np.asarray(Wp, np.float32)
    bp = np.asarray(bp, np.float32)
    W_ih = np.asarray(W_ih, np.float32)
    W_hh = np.asarray(W_hh, np.float32)
    b_ih = np.asarray(b_ih, np.float32)
    b_hh = np.asarray(b_hh, np.float32)
    initial_weight = np.asarray(initial_weight, np.float32)
    b_gcn = np.asarray(b_gcn, np.float32)
    assert x.shape == (N_NODES, IN_DIM)

    pre = _preprocess(edge_index)
    dinv, perm, s2 = pre["dinv"], pre["perm"], pre["s2"]
    nidx = pre["nidx"]

    key = tuple(int(v) for v in nidx.ravel())
    if key not in _cache:
        _cache[key] = _build_module(
            [[int(nidx[g][k]) for k in range(NCHK)] for g in range(NGRP)])
    nc = _cache[key]

    # shared tensors: host pre-scales x rows by dinv and casts to fp16
    xp = np.zeros((NP, IN_DIM), np.float32)
    xp[:N_NODES] = x * dinv[:, None]
    xs_tiled = np.ascontiguousarray(
        xp.reshape(RANKS, 128, IN_DIM).transpose(1, 0, 2)
        .reshape(128, RANKS * 128)).astype(np.float16)
    iota_np = np.tile(np.arange(128, dtype=np.float16)[None, :], (128, 1))
    shared = {
        "iota": np.ascontiguousarray(iota_np),
        "W_ihT": np.ascontiguousarray(W_ih.T),
        "W_hhT": np.ascontiguousarray(W_hh.T),
        "IW": initial_weight,
        "IWT": np.ascontiguousarray(initial_weight.T),
        "WpT": np.ascontiguousarray(Wp.T),
        "b_ih": b_ih.reshape(1, -1),
        "b_hh": b_hh.reshape(1, -1),
        "bp_col": np.ascontiguousarray(bp.reshape(-1, 1)),
        "b_gcn": b_gcn.reshape(1, -1),
        "ones_row": np.ones((1, 128), np.float32),
    }
    for k in range(NCHK):
        shared[f"xs{k}"] = np.ascontiguousarray(
            xs_tiled[:, k * CHTOK:(k + 1) * CHTOK])
    NLOC = N_NODES // M
    in_maps = []
    for c in range(M):
        pc = perm[c]
        s2p = np.zeros(NPC, np.float32)
        s2p[:NLOC] = s2[pc]
        drip = np.zeros(NPC, np.float32)
        drip[:NLOC] = 1.0 / dinv[pc]
        dlocp = np.zeros(NPC, np.float32)
        dlocp[:NLOC] = dinv[pc]
        in_maps.append({
            **shared,
            "idx": np.ascontiguousarray(pre["idx_rep"][c]),
            "dla": np.ascontiguousarray(pre["dla_t"][c]),
            "s2_row": s2p.reshape(1, -1),
            "dri_row": drip.reshape(1, -1),
            "dinv_col": np.ascontiguousarray(dlocp.reshape(NGRP, 128).T),
        })

    res = run_bass_kernel_spmd(nc, in_maps, list(range(M)))
    LAST_RESULT = res

    out = np.empty((N_NODES, HID), np.float32)
    for c in range(M):
        out[perm[c]] = res.results[c]["out"][:NLOC]
    return out
